# revision 23
# baseline (speedup 1.0000x reference)
"""DTW frames layer on 8 Trainium2 NeuronCores.

Reference computation (per (n, k) problem):
    cost[p, w] = max(0, ||x[n, :, w] - patts[k, :, p]||^2)          (P=32, W=128)
    dtw[0, w]  = cumsum_w cost[0, w]
    dtw[p, 0]  = cumsum_p cost[p, 0]
    dtw[p, w]  = cost[p, w] + min(dtw[p, w-1], dtw[p-1, w-1], dtw[p-1, w])
    out        = sqrt(dtw[:, -32:]) / 32

Strategy:
  - Data-parallel over batch n: each of the 8 cores owns n_loc = 8 rows of x,
    patterns replicated. Per core, two problem tiles of 128 partitions each
    (4 n x 32 k).
  - Cost matrix via one augmented K=10 fp32 matmul per (row-quad m, n-chunk):
    lhsT columns ordered p-major so PSUM partitions are (p%4, k); lhsT rows =
    [-2*patts[d], ||patt||^2, 1], rhs rows = [x[d], 1, ||x||^2], so PSUM is
    the cost before clamping; ReLU on eviction applies max(0, .).
  - Eviction collects all 8 row-quads into mm_big[128, (nn4, m8, w)]; the
    (p%4, k) -> (nn, k) partition regrouping is then a pure partition-block
    copy with identical free layout, done by 64 GPSIMD copies [32, 4, 128]
    (no DMA descriptors at all - the earlier DMA permute was descriptor-rate
    bound at ~90us).
  - DTW row recurrence on the DVE tensor_tensor_scan instruction:
    state = (m[t] min state) add c[t], one instruction per table row, where
    m[t] = min(dtw[p-1, t-1], dtw[p-1, t]) is one shifted tensor_tensor min.
    Column 0 of the m operand stays at +BIG so element 0 of each scan is
    initial + cost (the first-column cumsum), initial = prev row's column 0.
  - Rows land in a persistent D[128, P, W] buffer; one batched Sqrt
    activation per tile computes sqrt(dtw/1024 + 1e-6) = sqrt(dtw)/32 over
    the last 32 columns (the 1e-6 guards fp-rounding negatives; the clamp
    already ran on eviction).
  - Every ISA instruction has ONE sync-wait slot, so the build carefully
    keeps each instruction's emitted waits to a single semaphore: tiny
    same-engine "wait absorber" ops soak up producer waits ahead of DMA
    triggers and cross-engine consumers, and a tail nop-chain feeds every
    proc's final tick into the sync sequencer so the kernel-tail drain
    elides its (single-slot) wait list.
"""

import numpy as np

import concourse.bass as bass
import concourse.mybir as mybir
import concourse.tile as tile
from concourse.bass_utils import run_bass_kernel_spmd

N, D, W = 64, 8, 128      # x: (N, D, W)
K, P = 32, 32             # patts: (K, D, P)
WO = 32                   # output keeps last WO columns of the DTW table
NCORES = 8
NLOC = N // NCORES        # 8 batch rows per core
NT = 2                    # problem tiles per core: (4 n x 32 k) = 128 partitions
KAUG = D + 2              # augmented contraction dim
BIG = 1e30

f32 = mybir.dt.float32


def _rowmap(p: int) -> int:
    """C2 stores row p at index (p%4)*8 + p//4 (copy-contiguity order)."""
    return (p % 4) * 8 + p // 4


def build_program() -> bass.Bass:
    from concourse.tile import add_dep_helper

    nc = bass.Bass()
    inp_d = nc.dram_tensor("inp", (KAUG, K * P + NLOC * W), f32, kind="ExternalInput")
    out_d = nc.dram_tensor("out", (NLOC, K, P, WO), f32, kind="ExternalOutput")

    with tile.TileContext(nc) as tc:
        with (
            tc.tile_pool(name="consts", bufs=1) as consts,
            tc.tile_pool(name="psum", bufs=4, space="PSUM") as psum_pool,
            tc.tile_pool(name="mmb", bufs=1) as mmb_pool,
            tc.tile_pool(name="cbuf", bufs=1) as c_pool,
            tc.tile_pool(name="dbuf", bufs=1) as d_pool,
            tc.tile_pool(name="mbuf", bufs=2) as m_pool,
            tc.tile_pool(name="obuf", bufs=2) as o_pool,
        ):
            inp_s = consts.tile([KAUG, K * P + NLOC * W], f32)
            nc.sync.dma_start(out=inp_s, in_=inp_d[:, :])
            lhs_s = inp_s[:, 0:K * P]
            rhs_s = inp_s[:, K * P:K * P + NLOC * W]
            facta = consts.tile([1, 1], f32)
            factd = [
                consts.tile([1, 1], f32, name=f"factd{i}", tag=f"factd{i}")
                for i in range(2 * NT)
            ]

            # mm_big[t]: cost rows in matmul partition order (p%4, k), free
            # layout (nn, m, w). C2[t]: scan layout, partitions (nn, k),
            # rows stored in _rowmap order, free (p', w).
            mm_big = [
                mmb_pool.tile([128, 4, 8, W], f32, tag=f"mmb{t}", name=f"mmb{t}")
                for t in range(NT)
            ]
            C2 = [
                c_pool.tile([128, P, W], f32, tag=f"C{t}", name=f"C{t}")
                for t in range(NT)
            ]

            last_mm = None
            relus = {}
            for m in range(8):           # row-quad: p in {4m .. 4m+3}
                for t in range(NT):      # n-chunk: n in {4t .. 4t+3}
                    ps = psum_pool.tile([128, 512], f32)
                    last_mm = nc.tensor.matmul(
                        ps,
                        lhs_s[:, m * 128:(m + 1) * 128],
                        rhs_s[:, t * 512:(t + 1) * 512],
                        start=True,
                        stop=True,
                    )
                    relus[(m, t)] = nc.scalar.activation(
                        mm_big[t][:, :, m, :], ps,
                        mybir.ActivationFunctionType.Relu,
                    )

            # Partition-block regroup: (pp, k) quads -> (nn, k) quads.
            copies = {}
            for t in range(NT):
                for mq in range(2):          # 4 row-quads per copy
                    for nn in range(4):
                        for pp in range(4):
                            cp = nc.gpsimd.tensor_copy(
                                C2[t][nn * 32:(nn + 1) * 32,
                                      pp * 8 + mq * 4:pp * 8 + mq * 4 + 4, :],
                                mm_big[t][pp * 32:(pp + 1) * 32,
                                          nn, mq * 4:(mq + 1) * 4, :],
                            )
                            copies[(t, mq, nn, pp)] = cp

            last_scan = []
            last_ofence = None
            odmas = []
            Ds = []
            for t in range(NT):
                mt = m_pool.tile([128, W], f32)
                nc.vector.memset(mt, BIG)
                Dt = d_pool.tile([128, P, W], f32, tag=f"D{t}", name=f"D{t}")
                Ds.append(Dt)
                scan = None
                for p in range(P):
                    if p % 16 == 0:
                        # DVE wait absorber: one sem wait covering the 16
                        # GPSIMD copies feeding rows [p, p+16).
                        mq = p // 16
                        df = nc.vector.tensor_copy(
                            factd[t * 2 + mq], C2[t][0:1, mq * 4, 0:1]
                        )
                        add_dep_helper(
                            df.ins, copies[(t, mq, 3, 3)].ins, sync=True,
                            reason="DVE absorbs copy batch",
                        )
                    cr = C2[t][:, _rowmap(p), :]
                    if p == 0:
                        scan = nc.vector.tensor_tensor_scan(
                            Dt[:, 0, :], mt, cr, 0.0,
                            mybir.AluOpType.min, mybir.AluOpType.add,
                        )
                        add_dep_helper(
                            scan.ins, df.ins, sync=False,
                            reason="first scan after DVE absorber",
                        )
                    else:
                        nc.vector.tensor_tensor(
                            mt[:, 1:W], Dt[:, p - 1, 0:W - 1], Dt[:, p - 1, 1:W],
                            mybir.AluOpType.min,
                        )
                        scan = nc.vector.tensor_tensor_scan(
                            Dt[:, p, :], mt, cr, Dt[:, p - 1, 0:1],
                            mybir.AluOpType.min, mybir.AluOpType.add,
                        )
                        if p == 16:
                            add_dep_helper(
                                scan.ins, df.ins, sync=False,
                                reason="scan 16 after DVE absorber",
                            )
                last_scan.append(scan)

                ot = o_pool.tile([128, P, WO], f32)
                nc.scalar.activation(
                    ot[:, :, :], Dt[:, :, W - WO:W],
                    mybir.ActivationFunctionType.Sqrt,
                    scale=1.0 / (P * P),
                )
                ofence = nc.scalar.activation(
                    facta, ot[0:1, P - 1, 0:1], mybir.ActivationFunctionType.Copy
                )
                last_ofence = ofence
                odma = nc.scalar.dma_start(
                    out=out_d[t * 4:(t + 1) * 4, :, :, :], in_=ot
                )
                add_dep_helper(
                    odma.ins, ofence.ins, sync=False,
                    reason="out DMA after ACT wait-absorber",
                )
                odmas.append(odma)

            # Tail: feed every proc's final tick into the sync sequencer so
            # the kernel-tail drain's single-slot wait list elides.
            tail_deps = odmas + [
                last_ofence, last_mm, copies[(NT - 1, 1, 3, 3)]
            ] + last_scan
            prev_nop = None
            for td in tail_deps:
                nop = nc.sync.nop()
                add_dep_helper(
                    nop.ins, td.ins, sync=True,
                    reason="drain pre-absorber: sync waits on proc tail",
                )
                if prev_nop is not None:
                    add_dep_helper(
                        nop.ins, prev_nop.ins, sync=False,
                        reason="keep nop chain ordered",
                    )
                prev_nop = nop
    return nc


def make_in_maps(x: np.ndarray, patts: np.ndarray) -> list[dict[str, np.ndarray]]:
    x = np.ascontiguousarray(x, dtype=np.float32)
    patts = np.ascontiguousarray(patts, dtype=np.float32)
    pf = patts.transpose(1, 2, 0).reshape(D, P * K)              # [d, (p k)]
    p2 = (patts * patts).sum(axis=1).T.reshape(1, P * K)         # [(p k)]
    ones_pk = np.ones((1, P * K), np.float32)
    lhs = np.concatenate([-2.0 * pf, p2, ones_pk], axis=0).astype(np.float32)

    in_maps = []
    for c in range(NCORES):
        xs = x[c * NLOC:(c + 1) * NLOC]                          # (8, 8, 128)
        xf = xs.transpose(1, 0, 2).reshape(D, NLOC * W)          # [d, (n w)]
        x2 = (xs * xs).sum(axis=1).reshape(1, NLOC * W)          # [(n w)]
        ones_nw = np.ones((1, NLOC * W), np.float32)
        rhs = np.concatenate([xf, ones_nw, x2], axis=0).astype(np.float32)
        in_maps.append({"inp": np.concatenate([lhs, rhs], axis=1)})
    return in_maps


_program_cache: bass.Bass | None = None


def kernel(x: np.ndarray, patts: np.ndarray) -> np.ndarray:
    global _program_cache
    if _program_cache is None:
        _program_cache = build_program()
    nc = _program_cache
    in_maps = make_in_maps(x, patts)
    res = run_bass_kernel_spmd(nc, in_maps, list(range(NCORES)))
    return np.concatenate([r["out"] for r in res.results], axis=0)


if __name__ == "__main__":
    rng = np.random.default_rng(0)
    x = rng.standard_normal((N, D, W), dtype=np.float32)
    patts = rng.standard_normal((K, D, P), dtype=np.float32)
    out = kernel(x, patts)
    print(out.shape, out.dtype)


# revision 24
# speedup vs baseline: 1.6591x; 1.6591x over previous
"""DTW frames layer on 8 Trainium2 NeuronCores.

Reference computation (per (n, k) problem):
    cost[p, w] = max(0, ||x[n, :, w] - patts[k, :, p]||^2)          (P=32, W=128)
    dtw[0, w]  = cumsum_w cost[0, w]
    dtw[p, 0]  = cumsum_p cost[p, 0]
    dtw[p, w]  = cost[p, w] + min(dtw[p, w-1], dtw[p-1, w-1], dtw[p-1, w])
    out        = sqrt(dtw[:, -32:]) / 32

Strategy:
  - Data-parallel over batch n: each of the 8 cores owns n_loc = 8 rows of x,
    patterns replicated. Per core, two problem tiles of 128 partitions each
    (4 n x 32 k).
  - Cost matrix via one augmented K=10 fp32 matmul per (row-quad m, n-chunk):
    lhsT columns ordered p-major so PSUM partitions are (p%4, k); lhsT rows =
    [-2*patts[d], ||patt||^2, 1], rhs rows = [x[d], 1, ||x||^2], so PSUM is
    the cost before clamping; ReLU on eviction applies max(0, .) and packs
    row-quads into mm_big[128, (nn4 m8 w128)].
  - The (p%4, k) -> (nn, k) partition regroup into scan layout C2 is pure
    data movement with 2 KB contiguous runs on both sides; it runs as 64
    ACT-issued SBUF->SBUF DMAs of [32 partitions x 2 KB] (the naive permute
    had 512 B runs and was descriptor-rate bound at ~90 us; GPSIMD copies
    measured ~2 us each).
  - DTW row recurrence on the DVE tensor_tensor_scan instruction:
    state = (m[t] min state) add c[t], one instruction per table row, where
    m[t] = min(dtw[p-1, t-1], dtw[p-1, t]) is one shifted tensor_tensor min.
    Column 0 of the m operand stays at +BIG so element 0 of each scan is
    initial + cost (the first-column cumsum), initial = prev row's column 0.
    All operands are dense 2D slices - multi-dim APs measurably slow DVE.
  - Rows land in a persistent D[128, P*W] buffer; one batched Sqrt
    activation per tile computes sqrt(dtw/1024) = sqrt(dtw)/32 on the last
    32 columns of every row.
  - Every ISA instruction has ONE sync-wait slot, so the build keeps each
    instruction's emitted waits to a single semaphore: tiny same-engine
    "wait absorber" ops soak up producer waits ahead of DMA triggers and
    cross-engine consumers, and a tail nop-chain feeds every proc's final
    tick into the sync sequencer so the kernel-tail drain elides its
    (single-slot) wait list.
"""

import numpy as np

import concourse.bass as bass
import concourse.mybir as mybir
import concourse.tile as tile
from concourse.bass_utils import run_bass_kernel_spmd

N, D, W = 64, 8, 128      # x: (N, D, W)
K, P = 32, 32             # patts: (K, D, P)
WO = 32                   # output keeps last WO columns of the DTW table
NCORES = 8
NLOC = N // NCORES        # 8 batch rows per core
NT = 2                    # problem tiles per core: (4 n x 32 k) = 128 partitions
KAUG = D + 2              # augmented contraction dim
BIG = 1e30

f32 = mybir.dt.float32


def _rowmap(p: int) -> int:
    """C2 stores row p at index (p%4)*8 + p//4 (copy-contiguity order)."""
    return (p % 4) * 8 + p // 4


def build_program() -> bass.Bass:
    from concourse.tile import add_dep_helper

    nc = bass.Bass()
    inp_d = nc.dram_tensor("inp", (KAUG, K * P + NLOC * W), f32, kind="ExternalInput")
    out_d = nc.dram_tensor("out", (NLOC, K, P, WO), f32, kind="ExternalOutput")

    with tile.TileContext(nc) as tc:
        with (
            tc.tile_pool(name="consts", bufs=1) as consts,
            tc.tile_pool(name="psum", bufs=4, space="PSUM") as psum_pool,
            tc.tile_pool(name="mmb", bufs=1) as mmb_pool,
            tc.tile_pool(name="cbuf", bufs=1) as c_pool,
            tc.tile_pool(name="dbuf", bufs=1) as d_pool,
            tc.tile_pool(name="mbuf", bufs=2) as m_pool,
            tc.tile_pool(name="obuf", bufs=2) as o_pool,
        ):
            inp_s = consts.tile([KAUG, K * P + NLOC * W], f32)
            nc.sync.dma_start(out=inp_s, in_=inp_d[:, :])
            lhs_s = inp_s[:, 0:K * P]
            rhs_s = inp_s[:, K * P:K * P + NLOC * W]
            facta = consts.tile([1, 1], f32)
            factd = [
                consts.tile([1, 1], f32, name=f"factd{i}", tag=f"factd{i}")
                for i in range(8 * 2 * NT)
            ]

            # mm_big[t]: cost rows in matmul partition order (p%4, k), free
            # flat (nn, m, w). C2[t]: scan layout, partitions (nn, k), rows
            # in _rowmap order. D[t]: the DTW table, rows in true order.
            mm_big = [
                mmb_pool.tile([128, 4 * 8 * W], f32, tag=f"mmb{t}", name=f"mmb{t}")
                for t in range(NT)
            ]
            C2 = [
                c_pool.tile([128, P * W], f32, tag=f"C{t}", name=f"C{t}")
                for t in range(NT)
            ]

            last_mm = None
            relu = {}
            for m in range(8):           # row-quad: p in {4m .. 4m+3}
                for t in range(NT):      # n-chunk: n in {4t .. 4t+3}
                    ps = psum_pool.tile([128, 512], f32)
                    last_mm = nc.tensor.matmul(
                        ps,
                        lhs_s[:, m * 128:(m + 1) * 128],
                        rhs_s[:, t * 512:(t + 1) * 512],
                        start=True,
                        stop=True,
                    )
                    # strided out: per nn a 128-elem w-run at stride 1024
                    mmv = mm_big[t].rearrange("q (nn m w) -> q nn m w", nn=4, m=8)
                    relu[(m, t)] = nc.scalar.activation(
                        mmv[:, :, m, :], ps,
                        mybir.ActivationFunctionType.Relu,
                    )

            # Partition-block regroup (pp, k) -> (nn, k): 64 ACT-issued DMAs
            # of [32 partitions x 2 KB contiguous] each. An ACT "wait
            # absorber" per (t, mq) soaks the relu wait so every DMA carries
            # only its queue-predecessor wait (one ISA slot).
            dmas = []                    # in issue order, for queue tracking
            copies = {}
            for t in range(NT):
                for mq in range(2):
                    fence = nc.scalar.activation(
                        facta, mm_big[t][0:1, 0:1],
                        mybir.ActivationFunctionType.Copy,
                    )
                    for mr in range(4):
                        add_dep_helper(
                            fence.ins, relu[(mq * 4 + mr, t)].ins, sync=True,
                            reason="ACT absorbs relu batch",
                        )
                    for nn in range(4):
                        for pp in range(4):
                            dma = nc.scalar.dma_start(
                                out=C2[t][nn * 32:(nn + 1) * 32,
                                          (pp * 8 + mq * 4) * W:
                                          (pp * 8 + mq * 4 + 4) * W],
                                in_=mm_big[t][pp * 32:(pp + 1) * 32,
                                              nn * 1024 + mq * 512:
                                              nn * 1024 + (mq + 1) * 512],
                            )
                            add_dep_helper(
                                dma.ins, fence.ins, sync=False,
                                reason="regroup DMA after ACT absorber",
                            )
                            copies[(t, mq, nn, pp)] = dma
                            dmas.append(dma)

            last_scan = []
            last_ofence = None
            odmas = []
            for t in range(NT):
                mt = m_pool.tile([128, W], f32)
                nc.vector.memset(mt, BIG)
                Dt = d_pool.tile([128, P * W], f32, tag=f"D{t}", name=f"D{t}")
                scan = None
                first = None
                for p in range(P):
                    if p % 16 == 0:
                        # DVE wait absorbers: the 16 regroup DMAs feeding
                        # rows [p, p+16) span all 8 HWDGE queues; one
                        # single-wait fence per queue-max DMA (the last 8
                        # of the batch) covers them all.
                        mq = p // 16
                        batch = [copies[(t, mq, nn, pp)]
                                 for nn in range(4) for pp in range(4)]
                        for j, bd in enumerate(batch[8:]):
                            df = nc.vector.tensor_copy(
                                factd[(t * 2 + mq) * 8 + j],
                                C2[t][0:1, mq * 4 * W:mq * 4 * W + 1],
                            )
                            add_dep_helper(
                                df.ins, bd.ins, sync=True,
                                reason="DVE absorbs regroup queue",
                            )
                            first = df
                    r = _rowmap(p)
                    cr = C2[t][:, r * W:(r + 1) * W]
                    if p == 0:
                        scan = nc.vector.tensor_tensor_scan(
                            Dt[:, 0:W], mt, cr, 0.0,
                            mybir.AluOpType.min, mybir.AluOpType.add,
                        )
                        add_dep_helper(
                            scan.ins, first.ins, sync=False,
                            reason="first scan after DVE absorbers",
                        )
                    else:
                        o = (p - 1) * W
                        nc.vector.tensor_tensor(
                            mt[:, 1:W], Dt[:, o:o + W - 1], Dt[:, o + 1:o + W],
                            mybir.AluOpType.min,
                        )
                        scan = nc.vector.tensor_tensor_scan(
                            Dt[:, p * W:(p + 1) * W], mt, cr, Dt[:, o:o + 1],
                            mybir.AluOpType.min, mybir.AluOpType.add,
                        )
                        if p == 16:
                            add_dep_helper(
                                scan.ins, first.ins, sync=False,
                                reason="scan 16 after DVE absorbers",
                            )
                last_scan.append(scan)

                ot = o_pool.tile([128, P, WO], f32)
                dv = Dt.rearrange("q (p w) -> q p w", p=P)
                nc.scalar.activation(
                    ot[:, :, :], dv[:, :, W - WO:W],
                    mybir.ActivationFunctionType.Sqrt,
                    scale=1.0 / (P * P),
                )
                ofence = nc.scalar.activation(
                    facta, ot[0:1, P - 1, 0:1], mybir.ActivationFunctionType.Copy
                )
                last_ofence = ofence
                odma = nc.scalar.dma_start(
                    out=out_d[t * 4:(t + 1) * 4, :, :, :], in_=ot
                )
                add_dep_helper(
                    odma.ins, ofence.ins, sync=False,
                    reason="out DMA after ACT wait-absorber",
                )
                odmas.append(odma)

            # Tail: feed every proc's final tick into the sync sequencer so
            # the kernel-tail drain's single-slot wait list elides. The last
            # 8 regroup DMAs + the out DMAs cover every HWDGE queue's max.
            tail_deps = dmas[-8:] + odmas + [last_ofence, last_mm] + last_scan
            prev_nop = None
            for td in tail_deps:
                nop = nc.sync.nop()
                add_dep_helper(
                    nop.ins, td.ins, sync=True,
                    reason="drain pre-absorber: sync waits on proc tail",
                )
                if prev_nop is not None:
                    add_dep_helper(
                        nop.ins, prev_nop.ins, sync=False,
                        reason="keep nop chain ordered",
                    )
                prev_nop = nop
    return nc


def make_in_maps(x: np.ndarray, patts: np.ndarray) -> list[dict[str, np.ndarray]]:
    x = np.ascontiguousarray(x, dtype=np.float32)
    patts = np.ascontiguousarray(patts, dtype=np.float32)
    pf = patts.transpose(1, 2, 0).reshape(D, P * K)              # [d, (p k)]
    p2 = (patts * patts).sum(axis=1).T.reshape(1, P * K)         # [(p k)]
    ones_pk = np.ones((1, P * K), np.float32)
    lhs = np.concatenate([-2.0 * pf, p2, ones_pk], axis=0).astype(np.float32)

    in_maps = []
    for c in range(NCORES):
        xs = x[c * NLOC:(c + 1) * NLOC]                          # (8, 8, 128)
        xf = xs.transpose(1, 0, 2).reshape(D, NLOC * W)          # [d, (n w)]
        x2 = (xs * xs).sum(axis=1).reshape(1, NLOC * W)          # [(n w)]
        ones_nw = np.ones((1, NLOC * W), np.float32)
        rhs = np.concatenate([xf, ones_nw, x2], axis=0).astype(np.float32)
        in_maps.append({"inp": np.concatenate([lhs, rhs], axis=1)})
    return in_maps


_program_cache: bass.Bass | None = None


def kernel(x: np.ndarray, patts: np.ndarray) -> np.ndarray:
    global _program_cache
    if _program_cache is None:
        _program_cache = build_program()
    nc = _program_cache
    in_maps = make_in_maps(x, patts)
    res = run_bass_kernel_spmd(nc, in_maps, list(range(NCORES)))
    return np.concatenate([r["out"] for r in res.results], axis=0)


if __name__ == "__main__":
    rng = np.random.default_rng(0)
    x = rng.standard_normal((N, D, W), dtype=np.float32)
    patts = rng.standard_normal((K, D, P), dtype=np.float32)
    out = kernel(x, patts)
    print(out.shape, out.dtype)
